# revision 1
# baseline (speedup 1.0000x reference)
"""MoE (top-2 of 8 experts, SwiGLU) kernel for 8 TRN2 NeuronCores.

Expert-parallel with a split AllToAll combine. Core e holds expert e's
weights resident in SBUF and computes rows only for tokens routed to e
(host-side gather). Each token's two expert contributions are summed on the
COMBINER core chosen per token (balanced across the pair). Compact layout
per core (slot space):

  [ sendA 8*SA | sendB 8*SB | keepA 8*SA | keepB 8*SB ]   (S = SA+SB)

SA/SB are multiples of 16 so every region is 128-slot tile aligned. Send
segment d holds rows this core computed for tokens whose combiner is core d
(segment e itself = self-pair tokens, which the A2A returns in place). Two
world AllToAlls - fired as soon as their send region is fully written, so
they hide under the remaining compute - move send segment d to core d; the
receive buffers are then row-aligned with the keep regions (keep group s
slot j <-> recv row s*SA+j), so the combine is a fused DVE op per keep tile:
out = psy*wcg + recv. Keep group e is all pads (combined with recv diag =
self rows). Pads carry wcg=0 and compute to exact zeros - no zero-init, no
indirect DMA, no T x H collective. The host scatters output rows back to
token positions (pure placement, no arithmetic).

Weight loads are split into h-halves and interleaved with the first block's
matmul chains so the PE starts ~14us in instead of waiting ~50us for the
full 17MB of weights. Matmul operands are bf16 (fp32 PSUM accumulation);
rel err vs the fp32 reference is ~5e-3.
"""

import numpy as np
import ml_dtypes

import jax
import concourse.bass as bass
import concourse.tile as tile
from concourse import bacc, mybir
from concourse.bass import ts

E, H, I, T, KTOP = 8, 2048, 1408, 4096, 2
NCORES = 8

BF16 = mybir.dt.bfloat16
F32 = mybir.dt.float32


def _build_a2a_moe(S, SA, h=H, i_sz=I, ncores=NCORES, use_a2a=True):
    """S = total A2A rows per (src,dst) segment, SA = rows routed through the
    first AllToAll; both multiples of 16. Inputs per core: hsTg [h, C] bf16,
    wg/wu [h,i] bf16, wd [i,h] bf16, wcg [C] f32. Output: out [8S, h] bf16
    laid out [keepA-combined 8*SA | keepB-combined 8*SB]."""
    SB = S - SA
    assert SA % 16 == 0 and SB % 16 == 0 and SA > 0 and SB >= 0
    NS = ncores * S
    NSA, NSB = ncores * SA, ncores * SB
    C = 2 * NS
    hc, ic2, nct = h // 128, i_sz // 128, C // 128
    hh = hc // 4  # h-chunk quarter

    blocks = []
    pos = 0
    while pos < C:
        nb = min(512, C - pos)
        blocks.append((pos, nb))
        pos += nb

    last_a_tile = NSA // 128 - 1
    last_b_tile = NS // 128 - 1 if NSB else None

    nc = bacc.Bacc("TRN2", target_bir_lowering=False, debug=False,
                   num_devices=ncores)
    hsTg = nc.declare_dram_parameter("hsTg", [h, C], BF16, isOutput=False).ap()
    wg = nc.declare_dram_parameter("wg", [h, i_sz], BF16, isOutput=False).ap()
    wu = nc.declare_dram_parameter("wu", [h, i_sz], BF16, isOutput=False).ap()
    wd = nc.declare_dram_parameter("wd", [i_sz, h], BF16, isOutput=False).ap()
    wcg = nc.declare_dram_parameter("wcg", [C], F32, isOutput=False).ap()
    out = nc.declare_dram_parameter("out", [NS, h], BF16, isOutput=True).ap()

    silu = mybir.ActivationFunctionType.Silu
    world = [list(range(ncores))]

    with tile.TileContext(nc) as tc:
        with (
            tc.tile_pool(name="wpool", bufs=1) as wpool,
            tc.tile_pool(name="hspool", bufs=2) as hspool,
            tc.tile_pool(name="apool", bufs=1) as apool,
            tc.tile_pool(name="stage", bufs=2) as stage,
            tc.tile_pool(name="ypool", bufs=3) as ypool,
            tc.tile_pool(name="rpool", bufs=3) as rpool,
            tc.tile_pool(name="pg", bufs=2, space="PSUM") as pg,
            tc.tile_pool(name="pu", bufs=2, space="PSUM") as pu,
            tc.tile_pool(name="py", bufs=4, space="PSUM") as py,
            tc.tile_pool(name="dram", bufs=1, space="DRAM") as dram,
        ):
            # block 0's hidden states first in the DMA queue, then weight
            # halves in the order the first matmul chains consume them.
            (pos0, nb0) = blocks[0]
            hs0 = hspool.tile([128, hc, nb0], BF16, tag="hst")
            nc.sync.dma_start(
                out=hs0[:],
                in_=hsTg[:, pos0:pos0 + nb0].rearrange("(c p) t -> p c t", p=128))

            wg_h = [wpool.tile([128, hh, i_sz], BF16, name=f"wg{i}",
                               tag=f"wg{i}") for i in range(4)]
            wu_h = [wpool.tile([128, hh, i_sz], BF16, name=f"wu{i}",
                               tag=f"wu{i}") for i in range(4)]
            for i in range(4):
                nc.sync.dma_start(
                    out=wg_h[i][:],
                    in_=wg[i * hh * 128:(i + 1) * hh * 128, :]
                    .rearrange("(c p) i -> p c i", p=128))
                nc.sync.dma_start(
                    out=wu_h[i][:],
                    in_=wu[i * hh * 128:(i + 1) * hh * 128, :]
                    .rearrange("(c p) i -> p c i", p=128))
            wd_sb = wpool.tile([128, ic2, h], BF16, tag="wd")
            nc.sync.dma_start(out=wd_sb[:], in_=wd.rearrange("(c p) j -> p c j", p=128))
            wcg_sb = wpool.tile([128, nct], F32, tag="wcg")
            nc.sync.dma_start(out=wcg_sb[:], in_=wcg.rearrange("(ct p) -> p ct", p=128))

            sendA = dram.tile([NSA, h], BF16, tag="sendA")
            recvA = dram.tile([NSA, h], BF16, tag="recvA")
            if NSB:
                sendB = dram.tile([NSB, h], BF16, tag="sendB")
                recvB = dram.tile([NSB, h], BF16, tag="recvB")

            for bi, (pos, nb) in enumerate(blocks):
                if bi == 0:
                    hs_t = hs0
                else:
                    hs_t = hspool.tile([128, hc, nb], BF16, tag="hst")
                    nc.sync.dma_start(
                        out=hs_t[:],
                        in_=hsTg[:, pos:pos + nb].rearrange("(c p) t -> p c t", p=128))

                aT = apool.tile([128, ic2, nb], BF16, tag="aT")
                for it in range(ic2):
                    psg = pg.tile([128, nb], F32, tag="psg")
                    psu = pu.tile([128, nb], F32, tag="psu")
                    for half in range(4):
                        for c in range(hh):
                            gc = half * hh + c
                            nc.tensor.matmul(
                                psg[:], lhsT=wg_h[half][:, c, ts(it, 128)],
                                rhs=hs_t[:, gc, :],
                                start=(gc == 0), stop=(gc == hc - 1))
                        for c in range(hh):
                            gc = half * hh + c
                            nc.tensor.matmul(
                                psu[:], lhsT=wu_h[half][:, c, ts(it, 128)],
                                rhs=hs_t[:, gc, :],
                                start=(gc == 0), stop=(gc == hc - 1))
                    sil = stage.tile([128, nb], F32, tag="sil")
                    nc.scalar.activation(out=sil[:], in_=psg[:], func=silu)
                    nc.vector.tensor_mul(aT[:, it, :], sil[:], psu[:])

                for ct in range(nb // 128):
                    gct = pos // 128 + ct
                    lo = gct * 128
                    if lo < NSA:
                        region, q = "sA", lo
                    elif lo < NS:
                        region, q = "sB", lo - NSA
                    elif lo < NS + NSA:
                        region, q = "kA", lo - NS
                    else:
                        region, q = "kB", lo - NS - NSA

                    rcv = None
                    if region in ("kA", "kB"):
                        rcv = rpool.tile([128, h], BF16, tag="rcv")
                        src = recvA if region == "kA" else recvB
                        nc.sync.dma_start(out=rcv[:], in_=src[q:q + 128, :])

                    y_sb = ypool.tile([128, h], BF16, tag="ysb")
                    for hb in range(h // 512):
                        psy = py.tile([128, 512], F32, tag="psy")
                        for c2 in range(ic2):
                            nc.tensor.matmul(psy[:],
                                             lhsT=aT[:, c2, ts(ct, 128)],
                                             rhs=wd_sb[:, c2, ts(hb, 512)],
                                             start=(c2 == 0),
                                             stop=(c2 == ic2 - 1))
                        if rcv is None:
                            nc.vector.tensor_scalar_mul(
                                y_sb[:, ts(hb, 512)], psy[:],
                                wcg_sb[:, gct:gct + 1])
                        else:
                            nc.vector.scalar_tensor_tensor(
                                out=y_sb[:, ts(hb, 512)], in0=psy[:],
                                scalar=wcg_sb[:, gct:gct + 1],
                                in1=rcv[:, ts(hb, 512)],
                                op0=mybir.AluOpType.mult,
                                op1=mybir.AluOpType.add)

                    if region == "sA":
                        nc.sync.dma_start(out=sendA[q:q + 128, :], in_=y_sb[:])
                    elif region == "sB":
                        nc.sync.dma_start(out=sendB[q:q + 128, :], in_=y_sb[:])
                    elif region == "kA":
                        nc.sync.dma_start(out=out[q:q + 128, :], in_=y_sb[:])
                    else:
                        nc.sync.dma_start(out=out[NSA + q:NSA + q + 128, :],
                                          in_=y_sb[:])

                    if use_a2a and gct == last_a_tile:
                        nc.gpsimd.collective_compute(
                            "AllToAll", mybir.AluOpType.bypass,
                            replica_groups=world,
                            ins=[sendA[:].opt()], outs=[recvA[:].opt()])
                    if use_a2a and gct == last_b_tile:
                        nc.gpsimd.collective_compute(
                            "AllToAll", mybir.AluOpType.bypass,
                            replica_groups=world,
                            ins=[sendB[:].opt()], outs=[recvB[:].opt()])

    nc.compile()
    return nc


class _Runner:
    """Compile once, execute many. Mirrors bass2jax.run_bass_via_pjrt's
    multi-core path but keeps the jitted callable (and device-resident
    inputs) alive so repeat executions skip XLA/NEFF compilation."""

    def __init__(self, nc, n_cores):
        from concourse import bass2jax, mybir as _mybir
        from jax.experimental.shard_map import shard_map
        from jax.sharding import Mesh, PartitionSpec

        bass2jax.install_neuronx_cc_hook()
        partition_name = (nc.partition_id_tensor.name
                          if nc.partition_id_tensor else None)

        in_names, out_names, out_avals, zero_outs = [], [], [], []
        for alloc in nc.m.functions[0].allocations:
            if not isinstance(alloc, _mybir.MemoryLocationSet):
                continue
            name = alloc.memorylocations[0].name
            if alloc.kind == "ExternalInput":
                if name != partition_name:
                    in_names.append(name)
            elif alloc.kind == "ExternalOutput":
                shape = tuple(alloc.tensor_shape)
                dtype = _mybir.dt.np(alloc.dtype)
                out_names.append(name)
                out_avals.append(jax.core.ShapedArray(shape, dtype))
                zero_outs.append(np.zeros(shape, dtype))
        self.n_params = len(in_names)
        self.param_names = list(in_names)
        self.out_names = out_names
        self.out_avals = out_avals
        self.n_cores = n_cores
        all_names = in_names + out_names
        if partition_name is not None:
            all_names.append(partition_name)

        def _body(*args):
            operands = list(args)
            if partition_name is not None:
                operands.append(bass2jax.partition_id_tensor())
            outs = bass2jax._bass_exec_p.bind(
                *operands,
                out_avals=tuple(out_avals),
                in_names=tuple(all_names),
                out_names=tuple(out_names),
                lowering_input_output_aliases=(),
                sim_require_finite=True,
                sim_require_nnan=True,
                nc=nc,
            )
            return tuple(outs)

        devices = jax.devices()[:n_cores]
        assert len(devices) == n_cores
        mesh = Mesh(np.asarray(devices), ("core",))
        n_ops = self.n_params + len(out_names)
        self._body = _body
        self._mesh = mesh
        self._in_specs = (PartitionSpec("core"),) * n_ops
        self._out_specs = (PartitionSpec("core"),) * len(out_names)
        self._fn = jax.jit(
            shard_map(_body, mesh=mesh,
                      in_specs=self._in_specs,
                      out_specs=self._out_specs,
                      check_rep=False),
            keep_unused=True)
        self._zeros = [
            np.zeros((n_cores * z.shape[0], *z.shape[1:]), z.dtype)
            for z in zero_outs
        ]
        self._dev_args = None

    def prepare(self, in_maps):
        """Stage concatenated inputs, sharded across cores so execution
        never reshards (resharding would ship bytes through the host)."""
        from jax.sharding import NamedSharding, PartitionSpec
        sh = NamedSharding(self._mesh, PartitionSpec("core"))
        concat = [
            np.concatenate([np.asarray(in_maps[c][name])
                            for c in range(self.n_cores)], axis=0)
            for name in self.param_names
        ]
        self._dev_args = [jax.device_put(a, sh) for a in concat + self._zeros]

    def execute(self):
        outs = self._fn(*self._dev_args)
        jax.block_until_ready(outs)
        return outs

    def execute_chain(self, k):
        """Issue k async executions back-to-back, block once at the end.
        Device-side queuing overlaps the per-dispatch host round-trip, so
        wall(k) = floor + k * hw_exec and the slope over k isolates
        hw_exec."""
        outs = None
        for _ in range(k):
            outs = self._fn(*self._dev_args)
        jax.block_until_ready(outs)
        return outs

    def run(self, in_maps):
        self.prepare(in_maps)
        outs = self.execute()
        return [
            {name: np.asarray(outs[i]).reshape(self.n_cores,
                                               *self.out_avals[i].shape)[c]
             for i, name in enumerate(self.out_names)}
            for c in range(self.n_cores)
        ]


def _dispatch_a2a(hidden_states, top_k_index, top_k_weights):
    """Host-side routing. Returns per-core in_maps (hsTg, wcg), (S, SA), and
    the assembly plan (token list per out row range)."""
    hs = np.asarray(hidden_states, dtype=np.float32)
    idx = np.asarray(top_k_index).astype(np.int64)
    tw = np.asarray(top_k_weights, dtype=np.float32)

    w = np.zeros((E, T), dtype=np.float32)
    tarange = np.arange(T)
    for k in range(KTOP):
        np.add.at(w, (idx[:, k], tarange), tw[:, k])

    a = np.minimum(idx[:, 0], idx[:, 1])
    b = np.maximum(idx[:, 0], idx[:, 1])
    pair_tokens = {}
    for t in range(T):
        pair_tokens.setdefault((int(a[t]), int(b[t])), []).append(t)

    # side[(x,y)] = (tokens x combines, tokens y combines); balanced split
    side = {}
    self_toks = [[] for _ in range(E)]
    for (x, y), toks in pair_tokens.items():
        if x == y:
            self_toks[x] = toks
        else:
            side[(x, y)] = (toks[0::2], toks[1::2])

    S = max(max((max(len(sa), len(sb)) for sa, sb in side.values()),
                default=0),
            max(len(s) for s in self_toks))
    S += (-S) % 16  # region tile alignment
    # uneven A2A split, both halves multiples of 16
    SA = max(16, 16 * round(0.6 * S / 16))
    SA = min(SA, S)
    if S - SA == 0 and S > 16:
        SA = S - 16
    NS = NCORES * S
    NSA = NCORES * SA
    C = 2 * NS

    def side_list(e, p):
        """tokens of pair {e,p} combined by e."""
        x, y = min(e, p), max(e, p)
        sx, sy = side.get((x, y), ([], []))
        return sx if e == x else sy

    hsT_bf = np.ascontiguousarray(hs.T).astype(ml_dtypes.bfloat16)
    in_maps, plans = [], []
    for e in range(E):
        cols = np.zeros(C, dtype=np.int64)
        wcg = np.zeros(C, dtype=np.float32)
        keep_lists = []
        for d in range(E):
            toks = self_toks[e] if d == e else side_list(d, e)
            ta, tb = toks[:SA], toks[SA:]
            for j, t in enumerate(ta):
                cols[d * SA + j] = t
                wcg[d * SA + j] = w[e, t]
            for j, t in enumerate(tb):
                cols[NSA + d * (S - SA) + j] = t
                wcg[NSA + d * (S - SA) + j] = w[e, t]
        for s in range(E):
            # diag group stays all-pads: self rows arrive via the A2A diag
            toks = [] if s == e else side_list(e, s)
            ta, tb = toks[:SA], toks[SA:]
            for j, t in enumerate(ta):
                cols[NS + s * SA + j] = t
                wcg[NS + s * SA + j] = w[e, t]
            for j, t in enumerate(tb):
                cols[NS + NSA + s * (S - SA) + j] = t
                wcg[NS + NSA + s * (S - SA) + j] = w[e, t]
            keep_lists.append(self_toks[e] if s == e else toks)
        in_maps.append({
            "hsTg": np.ascontiguousarray(hsT_bf[:, cols]),
            "wcg": wcg,
        })
        plans.append(keep_lists)
    return in_maps, (S, SA), plans


def _assemble_a2a(results, caps, plans):
    S, SA = caps
    SB = S - SA
    NSA = NCORES * SA
    full = np.empty((T, H), dtype=np.float32)
    for e in range(NCORES):
        r_out = np.asarray(results[e]["out"], dtype=np.float32)
        for s in range(NCORES):
            toks = plans[e][s]
            ta, tb = toks[:SA], toks[SA:]
            if ta:
                full[np.asarray(ta, dtype=np.int64)] = \
                    r_out[s * SA:s * SA + len(ta)]
            if tb:
                full[np.asarray(tb, dtype=np.int64)] = \
                    r_out[NSA + s * SB:NSA + s * SB + len(tb)]
    return full


_RUNNERS = {}


def _get_runner(caps):
    if caps not in _RUNNERS:
        nc = _build_a2a_moe(caps[0], caps[1])
        _RUNNERS[caps] = _Runner(nc, NCORES)
    return _RUNNERS[caps]


def kernel(hidden_states, top_k_index, top_k_weights, Wg, Wu, Wd):
    in_maps, caps, plans = _dispatch_a2a(hidden_states, top_k_index,
                                         top_k_weights)
    for e in range(E):
        in_maps[e]["wg"] = np.asarray(Wg[e], dtype=np.float32).astype(ml_dtypes.bfloat16)
        in_maps[e]["wu"] = np.asarray(Wu[e], dtype=np.float32).astype(ml_dtypes.bfloat16)
        in_maps[e]["wd"] = np.asarray(Wd[e], dtype=np.float32).astype(ml_dtypes.bfloat16)
    runner = _get_runner(caps)
    results = runner.run(in_maps)
    return _assemble_a2a(results, caps, plans)

